# revision 22
# baseline (speedup 1.0000x reference)
"""BlockDWT2d (3-level Haar wavelet packet, 8x8 blocks) on 8 TRN2 NeuronCores.

Input  x: (32, 3, 512, 512) fp32 -> output (32, 192, 64, 64) fp32.

Math: the 3-level full packet transform is separable: for each 8x8 input
block, out2d = (H8/8) @ X8 @ H8^T where H8 is the natural-binary-order
Hadamard matrix; output channel k_sub bit-interleaves the row/col
transform indices (kH, kW): k_sub = 32h2+16w2+8h1+4w1+2h0+w0, and the
full channel index is K = 3*k_sub + c.

Per-core pipeline (batch-sharded 4 samples/core, 12 images of 512x512):
  DMA in:  X [p=h(128) x4 tiles, f=w(512)] (contiguous, GPSIMD SWDGE)
  Pass A   (per q: 4 mms t=0..3): psA[:, 128t:] = X[:,t,128q:]^T @ BD1
           BD1[(g,i),(kH,g')] = H8[kH,i]/8 * delta_gg'
           -> [p=(xbl,j), f=(t,kH,g)]; one copy/regroup -> Bq[p,(kH,yb)]
  Pass B   (per r: 4 mms q=0..3): psB[:, 128q:] = Bq[:, 128r:]^T @ BD2
           BD2[(xbl,j),(kW,xbl')] = H8[kW,j] * delta_xbl
           -> [p=(h0,yb), f=(q,kW,xbl)]; one copy/regroup -> Fr[p,(kW,xb)]
  DMA out: direct HWDGE DMAs, one per (img, r, kW): src [128p, 64f],
           DRAM dims (h0, yb, xb) — 3-dim AP, 256B runs. Alternated
           between the SP and ACT HWDGE rings.
"""

import numpy as np

_CACHE = {}


def _h8():
    x = np.eye(8, dtype=np.float32).reshape(1, 8, 8)
    for _ in range(3):
        a, b = x[:, 0::2, :], x[:, 1::2, :]
        x = np.concatenate([a + b, a - b], axis=0)
    return x[:, 0, :]  # H8[k, i], entries +-1


def _interleave(kH, kW):
    h2, h1, h0 = (kH >> 2) & 1, (kH >> 1) & 1, kH & 1
    w2, w1, w0 = (kW >> 2) & 1, (kW >> 1) & 1, kW & 1
    return 32 * h2 + 16 * w2 + 8 * h1 + 4 * w1 + 2 * h0 + 1 * w0


def _constants(bf16=False):
    H8 = _h8()
    bd1 = np.zeros((128, 128), np.float32)  # [(g,i), (kH,g')]
    for g in range(16):
        bd1[g * 8:(g + 1) * 8, :].reshape(8, 8, 16)[:, :, g] = (H8.T / 8.0)
    bd2 = np.zeros((128, 128), np.float32)  # [(xbl,j), (kW,xbl')]
    for xbl in range(16):
        bd2[xbl * 8:(xbl + 1) * 8, :].reshape(8, 8, 16)[:, :, xbl] = H8.T
    if bf16:
        import ml_dtypes
        # +-1 and +-1/8 are exact in bf16
        bd1 = bd1.astype(ml_dtypes.bfloat16)
        bd2 = bd2.astype(ml_dtypes.bfloat16)
    return {"bd1": bd1, "bd2": bd2}


# Host-side channel reindex for the v7 permuted dump:
# device channel order is (kH, kW, c); true K = 3*interleave(kH,kW) + c.
def _v7_src_of():
    src = np.zeros(192, np.int64)
    for kH in range(8):
        for kW in range(8):
            for c in range(3):
                src[3 * _interleave(kH, kW) + c] = (kH * 8 + kW) * 3 + c
    return src


_V7_SRC_OF = _v7_src_of()


def _postprocess(arr, variant="v7"):
    """Device dump [12, 128, 2048] -> [4, 192, 64, 64] (pure permutation,
    plus exact bf16->fp32 upcast for v8).

    Device layout: p = (kH8, u16), f = (t4, q4, kW8, v16);
    yb = t*16 + u, xb = q*16 + v, K = 3*interleave(kH,kW) + c.
    """
    if "v7" not in variant and "v8" not in variant:
        return arr
    if arr.dtype != np.float32:
        arr = arr.astype(np.float32)
    o = arr.reshape(4, 3, 8, 16, 4, 4, 8, 16)  # b c kH u t q kW v
    o = o.transpose(0, 2, 6, 1, 4, 3, 5, 7)    # b kH kW c t u q v
    o = np.ascontiguousarray(o).reshape(4, 192, 64, 64)
    return o[:, _V7_SRC_OF]


def _build_body_v4(nc, variant, x_in, out_v, bd1_s, bd2_s,
                   xpool, bpool, fpool, ppool, ppoolb, mybir, rep=0):
    """Pass-B M=64 (p=yb only); stage whole K-halves; 2 out-DMAs per sample.

    out_v: [4, 192, 64, 64] AP (ExternalOutput or scratch).
    Staging S_{b,h2} [64p(yb), f=(Klocal 96, xb 64)]; DMA dims
    [yb][K][xb]. Copy dst via 8-dim rearrange view.
    """
    do_out = "noout" not in variant
    in_eng = nc.sync if "insync" in variant else nc.gpsimd
    ncopy = 0
    for b in range(4):
        bqs_c = []
        for c in range(3):
            xt = xpool.tile([128, 4, 512], mybir.dt.float32, tag="x",
                            name=f"x_{rep}_{b}_{c}")
            in_eng.dma_start(
                xt[:], x_in.ap()[b * 3 + c].rearrange("(t p) w -> p t w",
                                                      p=128))
            bqs = []
            for q in range(4):
                psa = ppool.tile([128, 512], mybir.dt.float32, tag="ps",
                                 name=f"psA_{rep}_{b}_{c}_{q}")
                for t in range(4):
                    nc.tensor.matmul(
                        psa[:, t * 128:(t + 1) * 128],
                        lhsT=xt[:, t, q * 128:(q + 1) * 128],
                        rhs=bd1_s[:], start=True, stop=True)
                bq = bpool.tile([128, 512], mybir.dt.float32, tag="bq",
                                name=f"bq_{rep}_{b}_{c}_{q}")
                dst = bq.rearrange("p (a t g) -> p t a g", a=8, t=4)
                src = psa.rearrange("p (t a g) -> p t a g", t=4, a=8)
                if ncopy % 2 == 0:
                    nc.vector.tensor_copy(dst, src)
                else:
                    nc.scalar.copy(dst, src)
                ncopy += 1
                bqs.append(bq)
            bqs_c.append(bqs)
        for h2 in range(2):
            st = fpool.tile([128, 6144], mybir.dt.float32, tag="st",
                            name=f"st_{rep}_{b}_{h2}")
            # [p, w2, h1, w1, h0, w0, c, q, xbl]
            sv = st.rearrange(
                "p (w2 h1 w1 h0 w0 c q z) -> p w2 h1 w1 h0 w0 c q z",
                w2=2, h1=2, w1=2, h0=2, w0=2, c=3, q=4)
            colt = "v5" in variant
            for c in range(3):
                for u in range(2 if colt else 4):
                    # v5: kl pair (2u, 2u+1) col-tiled into one [128, 512]
                    # psum: rows 0:64 = h0=0, 64:128 = h0=1 (h1 = u).
                    if colt:
                        psb = ppoolb.tile([128, 512], mybir.dt.float32,
                                          tag="psb",
                                          name=f"psB_{rep}_{b}_{h2}_{c}_{u}")
                        for q in range(4):
                            for h0 in range(2):
                                kH = 4 * h2 + 2 * u + h0
                                nc.tensor.matmul(
                                    psb[h0 * 64:(h0 + 1) * 64,
                                        q * 128:(q + 1) * 128],
                                    lhsT=bqs_c[c][q][:, kH * 64:(kH + 1) * 64],
                                    rhs=bd2_s[:], start=True, stop=True)
                        pv = psb.rearrange(
                            "p (q w2 w1 w0 z) -> p q w2 w1 w0 z",
                            q=4, w2=2, w1=2, w0=2)
                        for h0 in range(2):
                            for w2 in range(2):
                                for w1 in range(2):
                                    src = pv[h0 * 64:(h0 + 1) * 64,
                                             :, w2, w1, :, :]
                                    dst = sv[:64, w2, u, w1, h0, :, c, :, :] \
                                        .transpose([0, 2, 1, 3])
                                    if ncopy % 2 == 0:
                                        nc.vector.tensor_copy(dst, src)
                                    else:
                                        nc.scalar.copy(dst, src)
                                    ncopy += 1
                        continue
                    kl = u
                    h1, h0 = kl // 2, kl % 2
                    kH = 4 * h2 + kl
                    psb = ppoolb.tile([64, 512], mybir.dt.float32,
                                      tag="psb",
                                      name=f"psB_{rep}_{b}_{h2}_{c}_{kl}")
                    for q in range(4):
                        nc.tensor.matmul(
                            psb[:, q * 128:(q + 1) * 128],
                            lhsT=bqs_c[c][q][:, kH * 64:(kH + 1) * 64],
                            rhs=bd2_s[:], start=True, stop=True)
                    # psb f = (q, w2, w1, w0, xbl); copy per (w2, w1):
                    pv = psb.rearrange(
                        "p (q w2 w1 w0 z) -> p q w2 w1 w0 z",
                        q=4, w2=2, w1=2, w0=2)
                    for w2 in range(2):
                        for w1 in range(2):
                            src = pv[:, :, w2, w1, :, :]  # (p, q, w0, z)
                            dst = sv[:64, w2, h1, w1, h0, :, c, :, :] \
                                .transpose([0, 2, 1, 3])  # (p, q, w0, z)
                            if ncopy % 2 == 0:
                                nc.vector.tensor_copy(dst, src)
                            else:
                                nc.scalar.copy(dst, src)
                            ncopy += 1
            if do_out:
                dma_dst = out_v[b][96 * h2:96 * (h2 + 1)].transpose([1, 0, 2])
                nc.sync.dma_start(dma_dst, st[:64, :])


def _build_body(nc, variant, x_in, ov, bd1_s, bd2_s,
                xpool, bpool, fpool, ppool, mybir, rep=0):
    do_mm = "dmaonly" not in variant
    do_out = "noout" not in variant
    ndma = 0
    for img in range(12):
        xt = xpool.tile([128, 4, 512], mybir.dt.float32, tag="x",
                        name=f"x_{rep}_{img}")
        nc.gpsimd.dma_start(
            xt[:], x_in.ap()[img].rearrange("(t p) w -> p t w", p=128))

        tmax = 1 if "mm1of4" in variant else 4
        bqs = []
        if do_mm:
            for q in range(4):
                psa = ppool.tile([128, 512], mybir.dt.float32, tag="ps",
                                 name=f"psA_{rep}_{img}_{q}")
                for t in range(tmax):
                    nc.tensor.matmul(psa[:, t * 128:(t + 1) * 128],
                                     lhsT=xt[:, t, q * 128:(q + 1) * 128],
                                     rhs=bd1_s[:], start=True, stop=True)
                bq = bpool.tile([128, 512], mybir.dt.float32, tag="bq",
                                name=f"bq_{rep}_{img}_{q}")
                dst = bq.rearrange("p (a t g) -> p t a g", a=8, t=4)
                src = psa.rearrange("p (t a g) -> p t a g", t=4, a=8)
                nc.vector.tensor_copy(dst, src)
                bqs.append(bq)

        b, c = img // 3, img % 3
        for r in range(4):
            h2, h1 = r // 2, r % 2
            fr = fpool.tile([128, 512], mybir.dt.float32, tag="fr",
                            name=f"fr_{rep}_{img}_{r}")
            if do_mm:
                psb = ppool.tile([128, 512], mybir.dt.float32, tag="ps",
                                 name=f"psB_{rep}_{img}_{r}")
                for q in range(tmax):
                    nc.tensor.matmul(psb[:, q * 128:(q + 1) * 128],
                                     lhsT=bqs[q][:, r * 128:(r + 1) * 128],
                                     rhs=bd2_s[:], start=True, stop=True)
                dst = fr.rearrange("p (a q g) -> p q a g", a=8, q=4)
                src = psb.rearrange("p (q a g) -> p q a g", q=4, a=8)
                nc.vector.tensor_copy(dst, src)
            else:
                nc.vector.tensor_copy(fr[:], xt[:, r, :])

            if do_out:
                for kw in range(8):
                    w2, w1, w0 = kw // 4, (kw // 2) % 2, kw % 2
                    # dst dims (h0, yb, xb) matches src enumeration
                    dma_dst = ov[b, h2, w2, h1, w1, :, w0, c, :, :]
                    eng = nc.sync if ndma % 2 == 0 else nc.scalar
                    eng.dma_start(dma_dst, fr[:, kw * 64:(kw + 1) * 64])
                    ndma += 1


def _build_body_v7(nc, variant, x_in, out_v, bd1_s, bd2_s,
                   xpool, bpool, fpool, ppool, ppoolb, mybir, rep=0):
    """v7: bf16 matmuls (weights exact in bf16), cast-on-DMA input,
    full-width (M=128) pass B, contiguous permuted output dump.

    Per img=(b,c):
      in:   gpsimd cast-DMA fp32->bf16, xtb[p=(u4,i3), f=(t2,w9)]
      A:    16 mm -> psa_q[p=(v4,j3), f=(t2,kH3,u4)] (4 psum banks)
      rgA:  4 copies (cast->bf16) -> bq_q[p, f=(ul2,kH3,t2,uh2)]
      B:    16 mm lhsT=bq_q[:,l*128:], rhs=bd2 -> psb_l[p=(kH,t,uh),
            f=(q2,kW3,v4)] (4 banks)
      stB:  4 contiguous copies -> st[p, f=(ul,q,kW,v)]
      out:  1 DMA st -> outp[img] (contiguous, 8KB/partition)
    Host-side: bit-interleave channel reindex (pure permutation).
    """
    insync = "insync" in variant
    ncopy = 0
    for img in range(12):
        xtb = xpool.tile([128, 4, 512], mybir.dt.bfloat16, tag="x",
                         name=f"x_{rep}_{img}")
        in_eng = nc.sync if insync else nc.gpsimd
        in_eng.dma_start(
            xtb[:], x_in.ap()[img].rearrange("(t p) w -> p t w", p=128))

        # Pass A: psa2_h packs q in {2h, 2h+1}; f = ql*512 + t*128 + (kh,u)
        bq2 = []
        for h in range(2):
            psa = ppool.tile([128, 1024], mybir.dt.float32, tag="ps",
                             name=f"psA_{rep}_{img}_{h}")
            for ql in range(2):
                q = 2 * h + ql
                for t in range(4):
                    nc.tensor.matmul(
                        psa[:, ql * 512 + t * 128:ql * 512 + (t + 1) * 128],
                        lhsT=xtb[:, t, q * 128:(q + 1) * 128],
                        rhs=bd1_s[:], start=True, stop=True)
            bq = bpool.tile([128, 1024], mybir.dt.bfloat16, tag="bq",
                            name=f"bq_{rep}_{img}_{h}")
            if ncopy % 2 == 0:
                nc.vector.tensor_copy(bq[:], psa[:])
            else:
                nc.scalar.copy(bq[:], psa[:])
            ncopy += 1
            bq2.append(bq)

        # Pass B: chunk select = t; psum p = (kh,u); psb2_g packs t in
        # {2g, 2g+1}; f = tl*512 + q*128 + (kW,v)
        odt = (mybir.dt.bfloat16 if "v8" in variant else mybir.dt.float32)
        st = fpool.tile([128, 2048], odt, tag="st",
                        name=f"st_{rep}_{img}")
        for g in range(2):
            psb = ppoolb.tile([128, 1024], mybir.dt.float32, tag="psb",
                              name=f"psB_{rep}_{img}_{g}")
            for tl in range(2):
                t = 2 * g + tl
                for q in range(4):
                    nc.tensor.matmul(
                        psb[:, tl * 512 + q * 128:tl * 512 + (q + 1) * 128],
                        lhsT=bq2[q // 2][:, (q % 2) * 512 + t * 128:
                                         (q % 2) * 512 + (t + 1) * 128],
                        rhs=bd2_s[:], start=True, stop=True)
            if ncopy % 2 == 0:
                nc.vector.tensor_copy(
                    st[:, g * 1024:(g + 1) * 1024], psb[:])
            else:
                nc.scalar.copy(st[:, g * 1024:(g + 1) * 1024], psb[:])
            ncopy += 1
        nc.sync.dma_start(out_v[img], st[:])


def _build_body_exp(nc, variant, x_in, out_d, cpool, xpool, mybir, reps):
    """DMA micro-experiments. Variants (combine with repN_ prefix):
      e_in     12 in-DMAs fp32 (gpsimd SWDGE)
      e_inc    12 in-DMAs casting fp32->bf16 (gpsimd)
      e_out    8 out-DMAs, 256B runs (v5 layout), sync HWDGE
      e_out2q  same split across sync+scalar HWDGE queues
      e_outc   4 contiguous out-DMAs (12288B runs), sync
      e_outc2q contiguous, split sync+scalar
      e_io     in fp32 + out 256B interleaved
      e_ioc    in cast + out contiguous interleaved
    """
    do_in = "e_in" in variant or "e_io" in variant
    cast = "e_inc" in variant or "e_ioc" in variant
    do_out = "e_out" in variant or "e_io" in variant
    contig = "e_outc" in variant or "e_ioc" in variant
    twoq = "2q" in variant

    st1 = st2 = None
    if do_out:
        if contig:
            st1 = cpool.tile([64, 12288], mybir.dt.float32, tag="stc1")
            nc.vector.memset(st1[:], 0.25)
        else:
            st1 = cpool.tile([64, 6144], mybir.dt.float32, tag="st1")
            st2 = cpool.tile([64, 6144], mybir.dt.float32, tag="st2")
            nc.vector.memset(st1[:], 0.25)
            nc.vector.memset(st2[:], 0.25)

    sink = cpool.tile([128, 8], mybir.dt.float32, tag="sink") if (
        "e_in" in variant or "e_io" in variant) else None
    last_xts = []
    for rep in range(reps):
        ndma = 0
        for img in range(12):
            if do_in:
                dt = mybir.dt.bfloat16 if cast else mybir.dt.float32
                xt = xpool.tile([128, 4, 512], dt, tag="x",
                                name=f"x_{rep}_{img}")
                nc.gpsimd.dma_start(
                    xt[:], x_in.ap()[img].rearrange("(t p) w -> p t w", p=128))
                last_xts.append(xt)
                last_xts = last_xts[-6:]
            if do_out:
                if contig and img < 4:
                    b = img
                    dst = out_d.ap()[b].rearrange(
                        "(a r) yb xb -> a (r yb xb)", a=64)
                    eng = nc.scalar if (twoq and ndma % 2) else nc.sync
                    eng.dma_start(dst, st1[:])
                    ndma += 1
                elif not contig and img < 8:
                    b, h2 = img // 2, img % 2
                    dst = out_d.ap()[b][96 * h2:96 * (h2 + 1)] \
                        .transpose([1, 0, 2])
                    eng = nc.scalar if (twoq and ndma % 2) else nc.sync
                    eng.dma_start(dst, (st1 if ndma % 2 else st2)[:])
                    ndma += 1
    # Observe in-DMA completion: copy a sliver of the final in-flight
    # tiles through the vector engine, then DMA the sink to the output.
    if sink is not None:
        for k, xt in enumerate(last_xts):
            nc.vector.tensor_copy(sink[:, k:k + 1], xt[:, 0, 0:1])
        nc.sync.dma_start(out_d.ap()[0][0][:16, :64], sink[:])


def _build_nc(variant="full"):
    from contextlib import ExitStack

    import concourse.tile as tile
    from concourse import bacc, mybir

    nc = bacc.Bacc("TRN2", target_bir_lowering=False, debug=False)

    v7 = "v7" in variant or "v8" in variant
    odtype = mybir.dt.bfloat16 if "v8" in variant else mybir.dt.float32
    cdtype = mybir.dt.bfloat16 if v7 else mybir.dt.float32
    x_in = nc.dram_tensor("x", [12, 512, 512], mybir.dt.float32,
                          kind="ExternalInput")
    bd1_d = nc.dram_tensor("bd1", [128, 128], cdtype,
                           kind="ExternalInput")
    bd2_d = nc.dram_tensor("bd2", [128, 128], cdtype,
                           kind="ExternalInput")
    out_shape = [12, 128, 2048] if v7 else [4, 192, 64, 64]
    out_d = nc.dram_tensor("out", out_shape, odtype,
                           kind="ExternalOutput")
    v4 = "v4" in variant or "v5" in variant
    exp = "e_" in variant
    with tile.TileContext(nc) as tc, ExitStack() as ctx:
        cpool = ctx.enter_context(tc.tile_pool(name="consts", bufs=1))
        xpool = ctx.enter_context(
            tc.tile_pool(name="xin",
                         bufs=6 if exp else (3 if v7 else (4 if v4 else 2))))
        bpool = ctx.enter_context(
            tc.tile_pool(name="bq", bufs=4 if v7 else (14 if v4 else 9)))
        fpool = ctx.enter_context(
            tc.tile_pool(name="fr", bufs=3 if (v4 or v7) else 9))
        ppool = ctx.enter_context(
            tc.tile_pool(name="ps", bufs=2 if v7 else (4 if v4 else 6),
                         space="PSUM"))
        ppoolb = (ctx.enter_context(
            tc.tile_pool(name="psb", bufs=2 if v7 else 4, space="PSUM"))
            if (v4 or v7) else None)

        bd1_s = cpool.tile([128, 128], cdtype, tag="bd1")
        bd2_s = cpool.tile([128, 128], cdtype, tag="bd2")
        nc.gpsimd.dma_start(bd1_s[:], bd1_d.ap())
        nc.gpsimd.dma_start(bd2_s[:], bd2_d.ap())

        if variant == "nop":
            nc.sync.dma_start(
                out_d.ap()[0, 0], bd1_s[:64, :64])
        else:
            reps = 1
            if variant == "double":
                reps = 2
            elif variant.startswith("rep"):
                reps = int(variant[3:].split("_")[0].replace("rep", "") or 1)
            outs_d = [out_d]
            for rep in range(1, reps):
                outs_d.append(nc.dram_tensor(
                    f"scr{rep}", out_shape, odtype))
            if exp:
                _build_body_exp(nc, variant, x_in, out_d, cpool, xpool,
                                mybir, reps)
                reps = 0
            for rep in range(reps):
                if v7:
                    _build_body_v7(nc, variant, x_in, outs_d[rep].ap(),
                                   bd1_s, bd2_s, xpool, bpool, fpool,
                                   ppool, ppoolb, mybir, rep=rep)
                elif v4:
                    _build_body_v4(nc, variant, x_in, outs_d[rep].ap(),
                                   bd1_s, bd2_s, xpool, bpool, fpool,
                                   ppool, ppoolb, mybir, rep=rep)
                else:
                    ovr = outs_d[rep].ap().rearrange(
                        "bb (h2 w2 h1 w1 h0 w0 c) yb xb -> "
                        "bb h2 w2 h1 w1 h0 w0 c yb xb",
                        h2=2, w2=2, h1=2, w1=2, h0=2, w0=2, c=3)
                    _build_body(nc, variant, x_in, ovr, bd1_s, bd2_s,
                                xpool, bpool, fpool, ppool, mybir, rep=rep)

    nc.compile()
    return nc


def _get_nc(variant="v5"):
    if variant not in _CACHE:
        _CACHE[variant] = _build_nc(variant)
    return _CACHE[variant]


VARIANT = "v8"


def kernel(x: np.ndarray) -> np.ndarray:
    from concourse.bass_utils import run_bass_kernel_spmd

    x = np.asarray(x, dtype=np.float32)
    assert x.shape == (32, 3, 512, 512)
    nc = _get_nc(VARIANT)
    consts = _constants(bf16="v7" in VARIANT or "v8" in VARIANT)
    in_maps = []
    for i in range(8):
        shard = np.ascontiguousarray(
            x[4 * i:4 * i + 4].reshape(12, 512, 512))
        in_maps.append({"x": shard, **consts})
    res = run_bass_kernel_spmd(nc, in_maps, core_ids=list(range(8)))
    return np.concatenate(
        [_postprocess(r["out"], VARIANT) for r in res.results], axis=0)

